# revision 13
# baseline (speedup 1.0000x reference)
"""LucidLinearAttention Trainium2 kernel (8-core SPMD), v3.

Sharding: batch b = core//2 (4 batches), head-group hg = core%2 (8 heads each).
Each core: qkv projection for its heads, chunked linear attention over
BT=512 blocks with exact BUCKET=64 causal masking inside the block, partial
output projection. Host sums the two head-group partials per batch.

v3 over v2:
- y stores + weight loads on the Activation HWDGE queue so the SP queue only
  carries x loads: next block's x prefetch is no longer stuck behind the
  current block's y stores (this was a ~4.75us bubble every block).
- Heads processed odds-first so the odd heads' SBUF->SBUF repartition DMAs
  (xot pair packing) complete while the even heads compute.
- S^T emitted 4 heads ahead of the OUT groups; per-head normalize tails
  (bcast/sbb/mul) deferred one head so PE never waits on the recip chain.
- Flexible PSUM->SBUF drains (masked S copies, sbb, ysb) greedily balanced
  across DVE and ACT by estimated cost.

v2 over baseline:
- f32r DRAM tensors, DMA straight into f32r SBUF (no staging copies).
- bf16 attention inner loop (S^T, intra/inter, transposes, C updates).
- Q projection pair-packed (M=128) and Y projection pair-packed (K=128);
  odd heads cross into the packed tiles' rows 64:127 via SBUF->SBUF DMA.
- S^T matmuls restricted to the needed query range per key chunk.
- C/kcum state in f32 (caug_st), re-rounded to bf16 operand each block.
- kcum initialized to 1e-30: den > 0 always, no clamp op needed.
"""
import sys
import numpy as np

for p in ("/opt/trn_rl_repo", "/root/.axon_site/_ro/trn_rl_repo"):
    if p not in sys.path:
        sys.path.insert(0, p)

import concourse.mybir as mybir
import concourse.tile as tile
from concourse import bacc
from concourse.bass_utils import run_bass_kernel_spmd
from concourse.masks import make_identity

F32 = mybir.dt.float32
F32R = mybir.dt.float32r
BF16 = mybir.dt.bfloat16
EXP = mybir.ActivationFunctionType.Exp

B, T, D = 4, 4096, 1024
NH, HD, BUCKET = 16, 64, 64
HPC = 8            # heads per core
GD = HPC * HD      # 512 group dim
NBLK = 8           # coarse blocks
BT = T // NBLK     # 512 rows per block
NPAIR = 4
NC_CORES = 8

_CACHE = {}


def _build():
    nc = bacc.Bacc("TRN2", target_bir_lowering=False, debug=False,
                   num_devices=NC_CORES)
    xT = nc.dram_tensor("xT", [D, T], F32R, kind="ExternalInput").ap()
    wqT = nc.dram_tensor("wqT", [D, GD], F32R, kind="ExternalInput").ap()
    wkT = nc.dram_tensor("wkT", [D, GD], F32R, kind="ExternalInput").ap()
    wvT = nc.dram_tensor("wvT", [D, GD], F32R, kind="ExternalInput").ap()
    woT = nc.dram_tensor("woT", [GD, D], F32R, kind="ExternalInput").ap()
    y = nc.dram_tensor("y", [T, D], F32, kind="ExternalOutput").ap()

    # greedy DVE/ACT balance for flexible PSUM->SBUF drains
    eng_acc = {"dve": 0.0, "act": 0.0}

    def flex_copy(dst, src, nfree):
        cd = 125 + 1.042 * nfree
        ca = 143 + 0.833 * nfree
        if eng_acc["dve"] + cd <= eng_acc["act"] + ca:
            eng_acc["dve"] += cd
            nc.vector.tensor_copy(dst, src)
        else:
            eng_acc["act"] += ca
            nc.scalar.copy(dst, src)

    def acc(engine, cost):
        eng_acc[engine] += cost

    with tile.TileContext(nc) as tc:
        with nc.allow_low_precision(reason="f32r/bf16 matmul rounding by design"), \
             tc.tile_pool(name="w", bufs=1) as wp, \
             tc.tile_pool(name="per", bufs=1) as pp, \
             tc.tile_pool(name="sb", bufs=1) as sbp, \
             tc.tile_pool(name="ps", bufs=1, space="PSUM") as ps:

            # ---- resident weights on the ACT HWDGE queue ----------------
            wq_sb = [wp.tile([128, GD], F32R, tag=f"wq{dc}", name=f"wq{dc}") for dc in range(8)]
            wk_sb = [wp.tile([128, GD], F32R, tag=f"wk{dc}", name=f"wk{dc}") for dc in range(8)]
            wv_sb = [wp.tile([128, GD], F32R, tag=f"wv{dc}", name=f"wv{dc}") for dc in range(8)]
            wo_sb = [wp.tile([128, D], F32R, tag=f"wo{p}", name=f"wo{p}") for p in range(NPAIR)]
            # scalar HWDGE queue: wq then wk (in block-0 use order);
            # sync queue stays free so block-0 x loads go first, then wv/wo
            # are appended behind them (emitted inside the ct==0 iteration).
            for dc in range(8):
                nc.scalar.dma_start(wq_sb[dc][:], wqT[128 * dc:128 * (dc + 1), :])
            for dc in range(8):
                nc.scalar.dma_start(wk_sb[dc][:], wkT[128 * dc:128 * (dc + 1), :])

            # ---- persistent state --------------------------------------
            ident_f = pp.tile([128, 128], F32, tag="ident_f")
            make_identity(nc, ident_f[:])
            ident_bf = pp.tile([128, 128], BF16, tag="ident_bf")
            nc.vector.tensor_copy(ident_bf[:], ident_f[:])
            bv_f32 = pp.tile([66, 64], F32, tag="bv_f32")
            nc.vector.memset(bv_f32[64:65, :], 1.0)
            bvec = pp.tile([66, 64], F32R, tag="bvec")
            nc.vector.tensor_copy(bvec[64:65, :], bv_f32[64:65, :])
            # bf16 copy of wo for the bf16 Y projection (filled after the
            # wo DMAs, which are emitted inside the ct==0 iteration)
            wo_bf = [pp.tile([128, D], BF16, tag=f"wo_bf{p}", name=f"wo_bf{p}")
                     for p in range(NPAIR)]
            # C/kcum state: f32 master + bf16 matmul operand
            caug_st = [pp.tile([64, 66], F32, tag=f"caug_st{h}", name=f"caug_st{h}")
                       for h in range(HPC)]
            for h in range(HPC):
                nc.gpsimd.memset(caug_st[h][:], 0.0)
                nc.gpsimd.memset(caug_st[h][:, 64:65], 1e-30)
            caug_bf = [pp.tile([128, 66], BF16, tag=f"caug_bf{h}", name=f"caug_bf{h}")
                       for h in range(HPC)]
            for h in range(HPC):
                nc.gpsimd.memset(caug_bf[h][:], 0.0)
                if h % 2 == 0:
                    nc.vector.tensor_copy(caug_bf[h][0:64, :], caug_st[h][:])
            for h in range(1, HPC, 2):
                cbs0 = pp.tile([64, 66], BF16, tag=f"caug_bfs{h}", name=f"cbs0_{h}")
                nc.vector.tensor_copy(cbs0[:], caug_st[h][:])
                nc.gpsimd.dma_start(caug_bf[h][64:128, :], cbs0[:])
            # vaug[s][t4]: [128, 528] bf16, 2 block-parity sets
            vaug = [[pp.tile([128, HPC * 66], BF16, tag=f"vaug{s}_{t}", name=f"vaug{s}_{t}")
                     for t in range(4)] for s in range(2)]
            for s in range(2):
                for t4 in range(4):
                    vv = vaug[s][t4][:].rearrange("p (h c) -> p h c", c=66)
                    nc.gpsimd.memset(vv[:, :, 64:65], 1.0)
                    nc.gpsimd.memset(vv[:, :, 65:66], 0.0)
            # ssb[q][t4]: masked S^T chunks, 4 head-parity sets (h%4), bf16
            ssb = [[pp.tile([128, BT], BF16, tag=f"ssb{q}_{t}", name=f"ssb{q}_{t}")
                    for t in range(4)] for q in range(4)]
            for q in range(4):
                for t4 in range(4):
                    nc.gpsimd.memset(ssb[q][t4][:], 0.0)

            # ---- per-block emission helpers ----------------------------
            HEAD_ORDER = [1, 3, 5, 7, 0, 2, 4, 6]

            def emit_st(h, q4, kt2, qtu2):
                """S^T chunks for head h, restricted query range + masked copies."""
                p, r = h // 2, h % 2
                rb = 64 * r
                for t4 in range(4):
                    c0 = (2 * t4 + 1) * 64
                    c1 = (2 * t4 + 2) * 64
                    pst = ps.tile([128, BT], F32, tag="s", name="pst", bufs=3)
                    nc.tensor.matmul(
                        pst[:, c0:BT],
                        kt2[p][rb:rb + 64, 128 * t4:128 * (t4 + 1)],
                        qtu2[p][rb:rb + 64, c0:BT], start=True, stop=True)
                    flex_copy(ssb[q4][t4][0:64, c0:BT], pst[0:64, c0:BT], BT - c0)
                    if c1 < BT:
                        flex_copy(ssb[q4][t4][64:128, c1:BT], pst[64:128, c1:BT],
                                  BT - c1)

            def emit_out(h, q4, qtu2, par2):
                """natural-orientation context: po_nat [128 tq, 4*66] f32.

                For query chunk j: cols 66j..66j+65 = [out (64) | den | pad].
                inter = qtu2 chunk (K=128, other head's rows killed by the
                zero half of caug_bf) @ caug_bf; intra i<=j = ssb[i] chunk j
                (K=128 keys) @ vaug[i] head cols.  All groups K=128.
                """
                p = h // 2
                po = ps.tile([128, 4 * 66], F32, tag="o", name="po", bufs=2)
                for j in range(4):
                    cj = 66 * j
                    nc.tensor.matmul(po[:, cj:cj + 66],
                                     qtu2[p][:, 128 * j:128 * (j + 1)],
                                     caug_bf[h][:, :], start=True, stop=False)
                    for i in range(j + 1):
                        nc.tensor.matmul(
                            po[:, cj:cj + 66],
                            ssb[q4][i][:, 128 * j:128 * (j + 1)],
                            vaug[par2][i][:, 66 * h:66 * h + 66],
                            start=False, stop=(i == j))
                return po

            def emit_norm(h, po):
                """per-partition normalize: dinv then xo (bf16) per chunk."""
                xo_sb = sbp.tile([128, 256], BF16, tag="xo_sb", name="xo_sb", bufs=4)
                for j in range(4):
                    dvn = sbp.tile([128, 1], F32, tag=f"dvn{j}", name=f"dvn{j}", bufs=4)
                    nc.vector.reciprocal(dvn[:], po[:, 66 * j + 64:66 * j + 65])
                    acc("dve", 130)
                    nc.vector.tensor_scalar_mul(
                        xo_sb[:, 64 * j:64 * (j + 1)],
                        po[:, 66 * j:66 * j + 64], dvn[:])
                    acc("dve", 192)
                return xo_sb

            def emit_xpose(h, xo_sb, xot2):
                """transpose xo chunks into the pair-packed Y operand."""
                p, r = h // 2, h % 2
                if r == 0:
                    dst, db = xot2[p], 0
                else:
                    dst = sbp.tile([64, BT], BF16, tag="xot_o", name="xot_o", bufs=2)
                    db = None
                for j in range(4):
                    ptx = ps.tile([64, 128], BF16, tag="s", name="ptx", bufs=3)
                    nc.tensor.transpose(ptx[:], xo_sb[:, 64 * j:64 * (j + 1)],
                                        ident_bf[:])
                    if r == 0:
                        flex_copy(dst[0:64, 128 * j:128 * (j + 1)], ptx[:], 128)
                    else:
                        flex_copy(dst[:, 128 * j:128 * (j + 1)], ptx[:], 128)
                if r == 1:
                    nc.gpsimd.dma_start(xot2[p][64:128, :], dst[:])

            def emit_cupd(h, ksb, par2, last=False):
                if last:
                    return
                pc = ps.tile([64, 66], F32, tag="c", name="pc", bufs=1)
                for t4 in range(4):
                    nc.tensor.matmul(
                        pc[:], ksb[t4][:, 64 * h:64 * (h + 1)],
                        vaug[par2][t4][:, 66 * h:66 * h + 66],
                        start=(t4 == 0), stop=(t4 == 3))
                nc.vector.tensor_add(caug_st[h][:], caug_st[h][:], pc[:])
                acc("dve", 194)
                if h % 2 == 0:
                    nc.vector.tensor_copy(caug_bf[h][0:64, :], caug_st[h][:])
                    acc("dve", 194)
                else:
                    cbs = pp.tile([64, 66], BF16, tag=f"caug_bfs{h}", name=f"cbs{h}")
                    nc.vector.tensor_copy(cbs[:], caug_st[h][:])
                    acc("dve", 194)
                    nc.gpsimd.dma_start(caug_bf[h][64:128, :], cbs[:])

            # ---- main loop over coarse blocks, software-pipelined ------
            # Iteration ct emits: x loads(ct) + interleaved [attention+Y of
            # block ct-1] and [projections of block ct].  PE then always has
            # independent projection matmuls available while the attention
            # dependency chains (ssb copies, recip/bcast/mul) resolve.
            prev = None
            for ct in range(NBLK + 1):
                proj_units = []
                if ct < NBLK:
                    t0 = ct * BT
                    par2 = ct % 2
                    xsb = [sbp.tile([128, BT], F32R, tag=f"xsb{dc}", name=f"xsb{dc}", bufs=2)
                           for dc in range(8)]
                    for dc in range(8):
                        nc.sync.dma_start(
                            xsb[dc][:], xT[128 * dc:128 * (dc + 1), t0:t0 + BT])
                    if ct == 0:
                        for dc in range(8):
                            nc.sync.dma_start(
                                wv_sb[dc][:], wvT[128 * dc:128 * (dc + 1), :])
                        for p in range(NPAIR):
                            nc.sync.dma_start(
                                wo_sb[p][:], woT[128 * p:128 * (p + 1), :])
                        for p in range(NPAIR):
                            nc.vector.tensor_copy(wo_bf[p][:], wo_sb[p][:])
                    qtu2 = [sbp.tile([128, BT], BF16, tag=f"qtu{p}", name=f"qtu{p}", bufs=2)
                            for p in range(NPAIR)]
                    ksb = [sbp.tile([128, GD], BF16, tag=f"ksb{t}", name=f"ksb{t}", bufs=2)
                           for t in range(4)]
                    kt2 = [sbp.tile([128, BT], BF16, tag=f"kt{p}", name=f"kt{p}", bufs=2)
                           for p in range(NPAIR)]

                    def mk_q(p, qtu2=qtu2, xsb=xsb):
                        def u():
                            pq = ps.tile([128, BT], F32, tag="proj", name="pq", bufs=2)
                            for dc in range(8):
                                nc.tensor.matmul(
                                    pq[:], wq_sb[dc][:, 128 * p:128 * (p + 1)], xsb[dc][:],
                                    start=(dc == 0), stop=(dc == 7))
                            nc.scalar.activation(qtu2[p][:], pq[:], EXP)
                            acc("act", 612)
                        return u

                    def mk_k(t4, ksb=ksb, xsb=xsb):
                        def u():
                            pk = ps.tile([128, GD], F32, tag="proj", name="pk", bufs=2)
                            for dc in range(8):
                                nc.tensor.matmul(
                                    pk[:], xsb[dc][:, 128 * t4:128 * (t4 + 1)], wk_sb[dc][:],
                                    start=(dc == 0), stop=(dc == 7))
                            nc.scalar.activation(ksb[t4][:], pk[:], EXP)
                            acc("act", 612)
                        return u

                    def mk_tr(t4, ksb=ksb, kt2=kt2):
                        # transpose chunk t4 for ALL pairs (reads only ksb[t4])
                        def u():
                            for p in range(NPAIR):
                                pt = ps.tile([128, 128], BF16, tag="s", name="pt", bufs=3)
                                nc.tensor.transpose(
                                    pt[:], ksb[t4][:, 128 * p:128 * (p + 1)], ident_bf[:])
                                flex_copy(kt2[p][:, 128 * t4:128 * (t4 + 1)], pt[:], 128)
                        return u

                    def mk_v(t4, xsb=xsb, par2=par2):
                        def u():
                            pv = ps.tile([128, GD], F32, tag="proj", name="pv", bufs=2)
                            for dc in range(8):
                                nc.tensor.matmul(
                                    pv[:], xsb[dc][:, 128 * t4:128 * (t4 + 1)], wv_sb[dc][:],
                                    start=(dc == 0), stop=(dc == 7))
                            vv = vaug[par2][t4][:].rearrange("p (h c) -> p h c", c=66)
                            pvv = pv[:].rearrange("p (h c) -> p h c", c=64)
                            flex_copy(vv[:, :, 0:64], pvv[:, :, :], BT)
                        return u

                    proj_units = [mk_q(0), mk_q(1), mk_q(2), mk_q(3),
                                  mk_k(0), mk_tr(0), mk_k(1), mk_tr(1),
                                  mk_k(2), mk_tr(2), mk_k(3), mk_tr(3),
                                  mk_v(0), mk_v(1), mk_v(2), mk_v(3)]
                    cur = dict(t0=t0, par2=par2, qtu2=qtu2, ksb=ksb, kt2=kt2)

                attn_units = []
                if prev is not None:
                    pv_t0, pv_par2 = prev["t0"], prev["par2"]
                    pv_qtu2, pv_ksb, pv_kt2 = prev["qtu2"], prev["ksb"], prev["kt2"]
                    xot2 = [sbp.tile([128, BT], BF16, tag=f"xot{p}", name=f"xot{p}", bufs=2)
                            for p in range(NPAIR)]
                    pend = []

                    def mk_st(i, kt2=pv_kt2, qtu2=pv_qtu2):
                        def u():
                            emit_st(HEAD_ORDER[i], i % 4, kt2, qtu2)
                        return u

                    last_blk = (ct == NBLK)

                    def mk_head(i, qtu2=pv_qtu2, ksb=pv_ksb, kt2=pv_kt2,
                                par2=pv_par2, xot2=xot2, pend=pend, last=last_blk):
                        def u():
                            h = HEAD_ORDER[i]
                            po = emit_out(h, i % 4, qtu2, par2)
                            if i + 4 < 8:
                                emit_st(HEAD_ORDER[i + 4], i % 4, kt2, qtu2)
                            xo_sb = emit_norm(h, po)
                            pend.append((h, xo_sb))
                            if len(pend) > 1:
                                emit_xpose(*pend.pop(0), xot2)
                            emit_cupd(h, ksb, par2, last=last)
                        return u

                    def mk_last_tail(pend=pend, xot2=xot2):
                        def u():
                            emit_xpose(*pend.pop(0), xot2)
                        return u

                    def mk_y(t4, fc, xot2=xot2, t0=pv_t0):
                        def u():
                            py = ps.tile([128, GD], F32, tag="proj", name="py", bufs=2)
                            for p in range(NPAIR):
                                nc.tensor.matmul(
                                    py[:],
                                    xot2[p][:, 128 * t4:128 * (t4 + 1)],
                                    wo_bf[p][:, GD * fc:GD * (fc + 1)],
                                    start=(p == 0), stop=(p == NPAIR - 1))
                            ysb = sbp.tile([128, GD], F32, tag="ysb", name="ysb", bufs=3)
                            flex_copy(ysb[:], py[:], GD)
                            nc.scalar.dma_start(
                                y[t0 + 128 * t4:t0 + 128 * (t4 + 1),
                                  GD * fc:GD * (fc + 1)], ysb[:])
                        return u

                    attn_units = ([mk_st(i) for i in range(4)]
                                  + [mk_head(i) for i in range(8)]
                                  + [mk_last_tail()]
                                  + [mk_y(t4, fc) for t4 in range(4) for fc in range(2)])

                # interleave: attention first (its deps are already met),
                # weaving projection units in proportionally
                na, np_ = len(attn_units), len(proj_units)
                if na == 0:
                    for u in proj_units:
                        u()
                else:
                    pi = 0
                    for k, u in enumerate(attn_units):
                        u()
                        want = (k + 1) * np_ // na
                        while pi < want:
                            proj_units[pi]()
                            pi += 1
                    while pi < np_:
                        proj_units[pi]()
                        pi += 1

                prev = cur if ct < NBLK else None

    nc.compile()
    return nc


def _get_nc():
    if "nc" not in _CACHE:
        _CACHE["nc"] = _build()
    return _CACHE["nc"]


def kernel(x, W_qkv, W_out):
    x = np.asarray(x, dtype=np.float32)
    W_qkv = np.asarray(W_qkv, dtype=np.float32)
    W_out = np.asarray(W_out, dtype=np.float32)
    nc = _get_nc()

    xTs = [np.ascontiguousarray(x[b].T) for b in range(B)]
    in_maps = []
    for c in range(NC_CORES):
        b, hg = c // 2, c % 2
        s = slice(hg * GD, (hg + 1) * GD)
        in_maps.append({
            "xT": xTs[b],
            "wqT": np.ascontiguousarray(W_qkv[0 * D:1 * D][s].T),
            "wkT": np.ascontiguousarray(W_qkv[1 * D:2 * D][s].T),
            "wvT": np.ascontiguousarray(W_qkv[2 * D:3 * D][s].T),
            "woT": np.ascontiguousarray(W_out[:, s].T),
        })
    res = run_bass_kernel_spmd(nc, in_maps, core_ids=list(range(NC_CORES)))
    out = np.empty((B, T, D), dtype=np.float32)
    for b in range(B):
        out[b] = res.results[2 * b]["y"] + res.results[2 * b + 1]["y"]
    return out


# revision 14
# speedup vs baseline: 1.0322x; 1.0322x over previous
"""LucidLinearAttention Trainium2 kernel (8-core SPMD), v3.

Sharding: batch b = core//2 (4 batches), head-group hg = core%2 (8 heads each).
Each core: qkv projection for its heads, chunked linear attention over
BT=512 blocks with exact BUCKET=64 causal masking inside the block, partial
output projection. Host sums the two head-group partials per batch.

v3 over v2:
- y stores + weight loads on the Activation HWDGE queue so the SP queue only
  carries x loads: next block's x prefetch is no longer stuck behind the
  current block's y stores (this was a ~4.75us bubble every block).
- Heads processed odds-first so the odd heads' SBUF->SBUF repartition DMAs
  (xot pair packing) complete while the even heads compute.
- S^T emitted 4 heads ahead of the OUT groups; per-head normalize tails
  (bcast/sbb/mul) deferred one head so PE never waits on the recip chain.
- Flexible PSUM->SBUF drains (masked S copies, sbb, ysb) greedily balanced
  across DVE and ACT by estimated cost.

v2 over baseline:
- f32r DRAM tensors, DMA straight into f32r SBUF (no staging copies).
- bf16 attention inner loop (S^T, intra/inter, transposes, C updates).
- Q projection pair-packed (M=128) and Y projection pair-packed (K=128);
  odd heads cross into the packed tiles' rows 64:127 via SBUF->SBUF DMA.
- S^T matmuls restricted to the needed query range per key chunk.
- C/kcum state in f32 (caug_st), re-rounded to bf16 operand each block.
- kcum initialized to 1e-30: den > 0 always, no clamp op needed.
"""
import sys
import numpy as np

for p in ("/opt/trn_rl_repo", "/root/.axon_site/_ro/trn_rl_repo"):
    if p not in sys.path:
        sys.path.insert(0, p)

import concourse.mybir as mybir
import concourse.tile as tile
from concourse import bacc
from concourse.bass_utils import run_bass_kernel_spmd
from concourse.masks import make_identity

F32 = mybir.dt.float32
F32R = mybir.dt.float32r
BF16 = mybir.dt.bfloat16
EXP = mybir.ActivationFunctionType.Exp

B, T, D = 4, 4096, 1024
NH, HD, BUCKET = 16, 64, 64
HPC = 8            # heads per core
GD = HPC * HD      # 512 group dim
NBLK = 8           # coarse blocks
BT = T // NBLK     # 512 rows per block
NPAIR = 4
NC_CORES = 8

_CACHE = {}


def _build():
    nc = bacc.Bacc("TRN2", target_bir_lowering=False, debug=False,
                   num_devices=NC_CORES)
    xT = nc.dram_tensor("xT", [D, T], F32R, kind="ExternalInput").ap()
    wqT = nc.dram_tensor("wqT", [D, GD], F32R, kind="ExternalInput").ap()
    wkT = nc.dram_tensor("wkT", [D, GD], F32R, kind="ExternalInput").ap()
    wvT = nc.dram_tensor("wvT", [D, GD], F32R, kind="ExternalInput").ap()
    woT = nc.dram_tensor("woT", [GD, D], F32R, kind="ExternalInput").ap()
    y = nc.dram_tensor("y", [T, D], F32, kind="ExternalOutput").ap()

    # greedy DVE/ACT balance for flexible PSUM->SBUF drains
    eng_acc = {"dve": 0.0, "act": 0.0}

    def flex_copy(dst, src, nfree):
        cd = 125 + 1.042 * nfree
        ca = (143 + 0.833 * nfree) * 1.35
        if eng_acc["dve"] + cd <= eng_acc["act"] + ca:
            eng_acc["dve"] += cd
            nc.vector.tensor_copy(dst, src)
        else:
            eng_acc["act"] += ca
            nc.scalar.copy(dst, src)

    def acc(engine, cost):
        eng_acc[engine] += cost

    with tile.TileContext(nc) as tc:
        with nc.allow_low_precision(reason="f32r/bf16 matmul rounding by design"), \
             tc.tile_pool(name="w", bufs=1) as wp, \
             tc.tile_pool(name="per", bufs=1) as pp, \
             tc.tile_pool(name="sb", bufs=1) as sbp, \
             tc.tile_pool(name="ps", bufs=1, space="PSUM") as ps:

            # ---- resident weights on the ACT HWDGE queue ----------------
            wq_sb = [wp.tile([128, GD], F32R, tag=f"wq{dc}", name=f"wq{dc}") for dc in range(8)]
            wk_sb = [wp.tile([128, GD], F32R, tag=f"wk{dc}", name=f"wk{dc}") for dc in range(8)]
            wv_sb = [wp.tile([128, GD], F32R, tag=f"wv{dc}", name=f"wv{dc}") for dc in range(8)]
            wo_sb = [wp.tile([128, D], F32R, tag=f"wo{p}", name=f"wo{p}") for p in range(NPAIR)]
            # scalar HWDGE queue: wq then wk (in block-0 use order);
            # sync queue stays free so block-0 x loads go first, then wv/wo
            # are appended behind them (emitted inside the ct==0 iteration).
            for dc in range(8):
                nc.scalar.dma_start(wq_sb[dc][:], wqT[128 * dc:128 * (dc + 1), :])
            for dc in range(8):
                nc.scalar.dma_start(wk_sb[dc][:], wkT[128 * dc:128 * (dc + 1), :])

            # ---- persistent state --------------------------------------
            ident_f = pp.tile([128, 128], F32, tag="ident_f")
            make_identity(nc, ident_f[:])
            ident_bf = pp.tile([128, 128], BF16, tag="ident_bf")
            nc.vector.tensor_copy(ident_bf[:], ident_f[:])
            bv_f32 = pp.tile([66, 64], F32, tag="bv_f32")
            nc.vector.memset(bv_f32[64:65, :], 1.0)
            bvec = pp.tile([66, 64], F32R, tag="bvec")
            nc.vector.tensor_copy(bvec[64:65, :], bv_f32[64:65, :])
            # bf16 copy of wo for the bf16 Y projection (filled after the
            # wo DMAs, which are emitted inside the ct==0 iteration)
            wo_bf = [pp.tile([128, D], BF16, tag=f"wo_bf{p}", name=f"wo_bf{p}")
                     for p in range(NPAIR)]
            # C/kcum state: f32 master + bf16 matmul operand
            caug_st = [pp.tile([64, 66], F32, tag=f"caug_st{h}", name=f"caug_st{h}")
                       for h in range(HPC)]
            for h in range(HPC):
                nc.gpsimd.memset(caug_st[h][:], 0.0)
                nc.gpsimd.memset(caug_st[h][:, 64:65], 1e-30)
            caug_bf = [pp.tile([128, 66], BF16, tag=f"caug_bf{h}", name=f"caug_bf{h}")
                       for h in range(HPC)]
            for h in range(HPC):
                nc.gpsimd.memset(caug_bf[h][:], 0.0)
                if h % 2 == 0:
                    nc.vector.tensor_copy(caug_bf[h][0:64, :], caug_st[h][:])
            for h in range(1, HPC, 2):
                cbs0 = pp.tile([64, 66], BF16, tag=f"caug_bfs{h}", name=f"cbs0_{h}")
                nc.vector.tensor_copy(cbs0[:], caug_st[h][:])
                nc.gpsimd.dma_start(caug_bf[h][64:128, :], cbs0[:])
            # vaug[s][t4]: [128, 528] bf16, 2 block-parity sets
            vaug = [[pp.tile([128, HPC * 66], BF16, tag=f"vaug{s}_{t}", name=f"vaug{s}_{t}")
                     for t in range(4)] for s in range(2)]
            for s in range(2):
                for t4 in range(4):
                    vv = vaug[s][t4][:].rearrange("p (h c) -> p h c", c=66)
                    nc.gpsimd.memset(vv[:, :, 64:65], 1.0)
                    nc.gpsimd.memset(vv[:, :, 65:66], 0.0)
            # ssb[q][t4]: masked S^T chunks, 4 head-parity sets (h%4), bf16
            ssb = [[pp.tile([128, BT], BF16, tag=f"ssb{q}_{t}", name=f"ssb{q}_{t}")
                    for t in range(4)] for q in range(4)]
            for q in range(4):
                for t4 in range(4):
                    nc.gpsimd.memset(ssb[q][t4][:], 0.0)

            # ---- per-block emission helpers ----------------------------
            HEAD_ORDER = [1, 3, 5, 7, 0, 2, 4, 6]

            def emit_st(h, q4, kt2, qtu2):
                """S^T chunks for head h, restricted query range + masked copies."""
                p, r = h // 2, h % 2
                rb = 64 * r
                for t4 in range(4):
                    c0 = (2 * t4 + 1) * 64
                    c1 = (2 * t4 + 2) * 64
                    pst = ps.tile([128, BT], F32, tag="s", name="pst", bufs=3)
                    nc.tensor.matmul(
                        pst[:, c0:BT],
                        kt2[p][rb:rb + 64, 128 * t4:128 * (t4 + 1)],
                        qtu2[p][rb:rb + 64, c0:BT], start=True, stop=True)
                    flex_copy(ssb[q4][t4][0:64, c0:BT], pst[0:64, c0:BT], BT - c0)
                    if c1 < BT:
                        flex_copy(ssb[q4][t4][64:128, c1:BT], pst[64:128, c1:BT],
                                  BT - c1)

            def emit_out(h, q4, qtu2, par2):
                """natural-orientation context: po_nat [128 tq, 4*66] f32.

                For query chunk j: cols 66j..66j+65 = [out (64) | den | pad].
                inter = qtu2 chunk (K=128, other head's rows killed by the
                zero half of caug_bf) @ caug_bf; intra i<=j = ssb[i] chunk j
                (K=128 keys) @ vaug[i] head cols.  All groups K=128.
                """
                p = h // 2
                po = ps.tile([128, 4 * 66], F32, tag="o", name="po", bufs=2)
                for j in range(4):
                    cj = 66 * j
                    nc.tensor.matmul(po[:, cj:cj + 66],
                                     qtu2[p][:, 128 * j:128 * (j + 1)],
                                     caug_bf[h][:, :], start=True, stop=False)
                    for i in range(j + 1):
                        nc.tensor.matmul(
                            po[:, cj:cj + 66],
                            ssb[q4][i][:, 128 * j:128 * (j + 1)],
                            vaug[par2][i][:, 66 * h:66 * h + 66],
                            start=False, stop=(i == j))
                return po

            def emit_norm(h, po):
                """per-partition normalize: dinv then xo (bf16) per chunk."""
                xo_sb = sbp.tile([128, 256], BF16, tag="xo_sb", name="xo_sb", bufs=4)
                for j in range(4):
                    dvn = sbp.tile([128, 1], F32, tag=f"dvn{j}", name=f"dvn{j}", bufs=4)
                    nc.vector.reciprocal(dvn[:], po[:, 66 * j + 64:66 * j + 65])
                    acc("dve", 130)
                    nc.vector.tensor_scalar_mul(
                        xo_sb[:, 64 * j:64 * (j + 1)],
                        po[:, 66 * j:66 * j + 64], dvn[:])
                    acc("dve", 192)
                return xo_sb

            def emit_xpose(h, xo_sb, xot2):
                """transpose xo chunks into the pair-packed Y operand."""
                p, r = h // 2, h % 2
                if r == 0:
                    dst, db = xot2[p], 0
                else:
                    dst = sbp.tile([64, BT], BF16, tag="xot_o", name="xot_o", bufs=2)
                    db = None
                for j in range(4):
                    ptx = ps.tile([64, 128], BF16, tag="s", name="ptx", bufs=3)
                    nc.tensor.transpose(ptx[:], xo_sb[:, 64 * j:64 * (j + 1)],
                                        ident_bf[:])
                    if r == 0:
                        flex_copy(dst[0:64, 128 * j:128 * (j + 1)], ptx[:], 128)
                    else:
                        flex_copy(dst[:, 128 * j:128 * (j + 1)], ptx[:], 128)
                if r == 1:
                    nc.gpsimd.dma_start(xot2[p][64:128, :], dst[:])

            def emit_cupd(h, ksb, par2, last=False):
                if last:
                    return
                pc = ps.tile([64, 66], F32, tag="c", name="pc", bufs=1)
                for t4 in range(4):
                    nc.tensor.matmul(
                        pc[:], ksb[t4][:, 64 * h:64 * (h + 1)],
                        vaug[par2][t4][:, 66 * h:66 * h + 66],
                        start=(t4 == 0), stop=(t4 == 3))
                nc.vector.tensor_add(caug_st[h][:], caug_st[h][:], pc[:])
                acc("dve", 194)
                if h % 2 == 0:
                    nc.vector.tensor_copy(caug_bf[h][0:64, :], caug_st[h][:])
                    acc("dve", 194)
                else:
                    cbs = pp.tile([64, 66], BF16, tag=f"caug_bfs{h}", name=f"cbs{h}")
                    nc.vector.tensor_copy(cbs[:], caug_st[h][:])
                    acc("dve", 194)
                    nc.gpsimd.dma_start(caug_bf[h][64:128, :], cbs[:])

            # ---- main loop over coarse blocks, software-pipelined ------
            # Iteration ct emits: x loads(ct) + interleaved [attention+Y of
            # block ct-1] and [projections of block ct].  PE then always has
            # independent projection matmuls available while the attention
            # dependency chains (ssb copies, recip/bcast/mul) resolve.
            prev = None
            for ct in range(NBLK + 1):
                proj_units = []
                if ct < NBLK:
                    t0 = ct * BT
                    par2 = ct % 2
                    xsb = [sbp.tile([128, BT], F32R, tag=f"xsb{dc}", name=f"xsb{dc}", bufs=2)
                           for dc in range(8)]
                    for dc in range(8):
                        nc.sync.dma_start(
                            xsb[dc][:], xT[128 * dc:128 * (dc + 1), t0:t0 + BT])
                    if ct == 0:
                        for dc in range(8):
                            nc.sync.dma_start(
                                wv_sb[dc][:], wvT[128 * dc:128 * (dc + 1), :])
                        for p in range(NPAIR):
                            nc.sync.dma_start(
                                wo_sb[p][:], woT[128 * p:128 * (p + 1), :])
                        for p in range(NPAIR):
                            nc.vector.tensor_copy(wo_bf[p][:], wo_sb[p][:])
                    qtu2 = [sbp.tile([128, BT], BF16, tag=f"qtu{p}", name=f"qtu{p}", bufs=2)
                            for p in range(NPAIR)]
                    ksb = [sbp.tile([128, GD], BF16, tag=f"ksb{t}", name=f"ksb{t}", bufs=2)
                           for t in range(4)]
                    kt2 = [sbp.tile([128, BT], BF16, tag=f"kt{p}", name=f"kt{p}", bufs=2)
                           for p in range(NPAIR)]

                    def mk_q(p, qtu2=qtu2, xsb=xsb):
                        def u():
                            pq = ps.tile([128, BT], F32, tag="proj", name="pq", bufs=2)
                            for dc in range(8):
                                nc.tensor.matmul(
                                    pq[:], wq_sb[dc][:, 128 * p:128 * (p + 1)], xsb[dc][:],
                                    start=(dc == 0), stop=(dc == 7))
                            nc.scalar.activation(qtu2[p][:], pq[:], EXP)
                            acc("act", 612)
                        return u

                    def mk_k(t4, ksb=ksb, xsb=xsb):
                        def u():
                            pk = ps.tile([128, GD], F32, tag="proj", name="pk", bufs=2)
                            for dc in range(8):
                                nc.tensor.matmul(
                                    pk[:], xsb[dc][:, 128 * t4:128 * (t4 + 1)], wk_sb[dc][:],
                                    start=(dc == 0), stop=(dc == 7))
                            nc.scalar.activation(ksb[t4][:], pk[:], EXP)
                            acc("act", 612)
                        return u

                    def mk_tr(t4, ksb=ksb, kt2=kt2):
                        # transpose chunk t4 for ALL pairs (reads only ksb[t4])
                        def u():
                            for p in range(NPAIR):
                                pt = ps.tile([128, 128], BF16, tag="s", name="pt", bufs=3)
                                nc.tensor.transpose(
                                    pt[:], ksb[t4][:, 128 * p:128 * (p + 1)], ident_bf[:])
                                flex_copy(kt2[p][:, 128 * t4:128 * (t4 + 1)], pt[:], 128)
                        return u

                    def mk_v(t4, xsb=xsb, par2=par2):
                        def u():
                            pv = ps.tile([128, GD], F32, tag="proj", name="pv", bufs=2)
                            for dc in range(8):
                                nc.tensor.matmul(
                                    pv[:], xsb[dc][:, 128 * t4:128 * (t4 + 1)], wv_sb[dc][:],
                                    start=(dc == 0), stop=(dc == 7))
                            vv = vaug[par2][t4][:].rearrange("p (h c) -> p h c", c=66)
                            pvv = pv[:].rearrange("p (h c) -> p h c", c=64)
                            flex_copy(vv[:, :, 0:64], pvv[:, :, :], BT)
                        return u

                    proj_units = [mk_q(0), mk_q(1), mk_q(2), mk_q(3),
                                  mk_k(0), mk_tr(0), mk_k(1), mk_tr(1),
                                  mk_k(2), mk_tr(2), mk_k(3), mk_tr(3),
                                  mk_v(0), mk_v(1), mk_v(2), mk_v(3)]
                    cur = dict(t0=t0, par2=par2, qtu2=qtu2, ksb=ksb, kt2=kt2)

                attn_units = []
                if prev is not None:
                    pv_t0, pv_par2 = prev["t0"], prev["par2"]
                    pv_qtu2, pv_ksb, pv_kt2 = prev["qtu2"], prev["ksb"], prev["kt2"]
                    xot2 = [sbp.tile([128, BT], BF16, tag=f"xot{p}", name=f"xot{p}", bufs=2)
                            for p in range(NPAIR)]
                    pend = []

                    def mk_st(i, kt2=pv_kt2, qtu2=pv_qtu2):
                        def u():
                            emit_st(HEAD_ORDER[i], i % 4, kt2, qtu2)
                        return u

                    last_blk = (ct == NBLK)

                    def mk_head(i, qtu2=pv_qtu2, ksb=pv_ksb, kt2=pv_kt2,
                                par2=pv_par2, xot2=xot2, pend=pend, last=last_blk):
                        def u():
                            h = HEAD_ORDER[i]
                            po = emit_out(h, i % 4, qtu2, par2)
                            if i + 4 < 8:
                                emit_st(HEAD_ORDER[i + 4], i % 4, kt2, qtu2)
                            xo_sb = emit_norm(h, po)
                            pend.append((h, xo_sb))
                            if len(pend) > 1:
                                emit_xpose(*pend.pop(0), xot2)
                            emit_cupd(h, ksb, par2, last=last)
                        return u

                    def mk_last_tail(pend=pend, xot2=xot2):
                        def u():
                            emit_xpose(*pend.pop(0), xot2)
                        return u

                    def mk_y(t4, fc, xot2=xot2, t0=pv_t0):
                        def u():
                            py = ps.tile([128, GD], F32, tag="proj", name="py", bufs=2)
                            for p in range(NPAIR):
                                nc.tensor.matmul(
                                    py[:],
                                    xot2[p][:, 128 * t4:128 * (t4 + 1)],
                                    wo_bf[p][:, GD * fc:GD * (fc + 1)],
                                    start=(p == 0), stop=(p == NPAIR - 1))
                            ysb = sbp.tile([128, GD], F32, tag="ysb", name="ysb", bufs=3)
                            flex_copy(ysb[:], py[:], GD)
                            nc.scalar.dma_start(
                                y[t0 + 128 * t4:t0 + 128 * (t4 + 1),
                                  GD * fc:GD * (fc + 1)], ysb[:])
                        return u

                    attn_units = ([mk_st(i) for i in range(4)]
                                  + [mk_head(i) for i in range(8)]
                                  + [mk_last_tail()]
                                  + [mk_y(t4, fc) for t4 in range(4) for fc in range(2)])

                # interleave: attention first (its deps are already met),
                # weaving projection units in proportionally
                na, np_ = len(attn_units), len(proj_units)
                if na == 0:
                    for u in proj_units:
                        u()
                else:
                    pi = 0
                    for k, u in enumerate(attn_units):
                        u()
                        want = (k + 1) * np_ // na
                        while pi < want:
                            proj_units[pi]()
                            pi += 1
                    while pi < np_:
                        proj_units[pi]()
                        pi += 1

                prev = cur if ct < NBLK else None

    nc.compile()
    return nc


def _get_nc():
    if "nc" not in _CACHE:
        _CACHE["nc"] = _build()
    return _CACHE["nc"]


def kernel(x, W_qkv, W_out):
    x = np.asarray(x, dtype=np.float32)
    W_qkv = np.asarray(W_qkv, dtype=np.float32)
    W_out = np.asarray(W_out, dtype=np.float32)
    nc = _get_nc()

    xTs = [np.ascontiguousarray(x[b].T) for b in range(B)]
    in_maps = []
    for c in range(NC_CORES):
        b, hg = c // 2, c % 2
        s = slice(hg * GD, (hg + 1) * GD)
        in_maps.append({
            "xT": xTs[b],
            "wqT": np.ascontiguousarray(W_qkv[0 * D:1 * D][s].T),
            "wkT": np.ascontiguousarray(W_qkv[1 * D:2 * D][s].T),
            "wvT": np.ascontiguousarray(W_qkv[2 * D:3 * D][s].T),
            "woT": np.ascontiguousarray(W_out[:, s].T),
        })
    res = run_bass_kernel_spmd(nc, in_maps, core_ids=list(range(NC_CORES)))
    out = np.empty((B, T, D), dtype=np.float32)
    for b in range(B):
        out[b] = res.results[2 * b]["y"] + res.results[2 * b + 1]["y"]
    return out


# revision 15
# speedup vs baseline: 1.0339x; 1.0016x over previous
"""LucidLinearAttention Trainium2 kernel (8-core SPMD), v3.

Sharding: batch b = core//2 (4 batches), head-group hg = core%2 (8 heads each).
Each core: qkv projection for its heads, chunked linear attention over
BT=512 blocks with exact BUCKET=64 causal masking inside the block, partial
output projection. Host sums the two head-group partials per batch.

v3 over v2:
- y stores + weight loads on the Activation HWDGE queue so the SP queue only
  carries x loads: next block's x prefetch is no longer stuck behind the
  current block's y stores (this was a ~4.75us bubble every block).
- Heads processed odds-first so the odd heads' SBUF->SBUF repartition DMAs
  (xot pair packing) complete while the even heads compute.
- S^T emitted 4 heads ahead of the OUT groups; per-head normalize tails
  (bcast/sbb/mul) deferred one head so PE never waits on the recip chain.
- Flexible PSUM->SBUF drains (masked S copies, sbb, ysb) greedily balanced
  across DVE and ACT by estimated cost.

v2 over baseline:
- f32r DRAM tensors, DMA straight into f32r SBUF (no staging copies).
- bf16 attention inner loop (S^T, intra/inter, transposes, C updates).
- Q projection pair-packed (M=128) and Y projection pair-packed (K=128);
  odd heads cross into the packed tiles' rows 64:127 via SBUF->SBUF DMA.
- S^T matmuls restricted to the needed query range per key chunk.
- C/kcum state in f32 (caug_st), re-rounded to bf16 operand each block.
- kcum initialized to 1e-30: den > 0 always, no clamp op needed.
"""
import sys
import ml_dtypes
import numpy as np

BF = ml_dtypes.bfloat16

for p in ("/opt/trn_rl_repo", "/root/.axon_site/_ro/trn_rl_repo"):
    if p not in sys.path:
        sys.path.insert(0, p)

import concourse.mybir as mybir
import concourse.tile as tile
from concourse import bacc
from concourse.bass_utils import run_bass_kernel_spmd
from concourse.masks import make_identity

F32 = mybir.dt.float32
F32R = mybir.dt.float32r
BF16 = mybir.dt.bfloat16
EXP = mybir.ActivationFunctionType.Exp

B, T, D = 4, 4096, 1024
NH, HD, BUCKET = 16, 64, 64
HPC = 8            # heads per core
GD = HPC * HD      # 512 group dim
NBLK = 8           # coarse blocks
BT = T // NBLK     # 512 rows per block
NPAIR = 4
NC_CORES = 8

_CACHE = {}


def _build():
    nc = bacc.Bacc("TRN2", target_bir_lowering=False, debug=False,
                   num_devices=NC_CORES)
    xT = nc.dram_tensor("xT", [D, T], BF16, kind="ExternalInput").ap()
    wqT = nc.dram_tensor("wqT", [D, GD], BF16, kind="ExternalInput").ap()
    wkT = nc.dram_tensor("wkT", [D, GD], BF16, kind="ExternalInput").ap()
    wvT = nc.dram_tensor("wvT", [D, GD], BF16, kind="ExternalInput").ap()
    woT = nc.dram_tensor("woT", [GD, D], BF16, kind="ExternalInput").ap()
    y = nc.dram_tensor("y", [T, D], F32, kind="ExternalOutput").ap()

    # greedy DVE/ACT balance for flexible PSUM->SBUF drains
    eng_acc = {"dve": 0.0, "act": 0.0}

    def flex_copy(dst, src, nfree):
        cd = 125 + 1.042 * nfree
        ca = (143 + 0.833 * nfree) * 1.35
        if eng_acc["dve"] + cd <= eng_acc["act"] + ca:
            eng_acc["dve"] += cd
            nc.vector.tensor_copy(dst, src)
        else:
            eng_acc["act"] += ca
            nc.scalar.copy(dst, src)

    def acc(engine, cost):
        eng_acc[engine] += cost

    with tile.TileContext(nc) as tc:
        with nc.allow_low_precision(reason="f32r/bf16 matmul rounding by design"), \
             tc.tile_pool(name="w", bufs=1) as wp, \
             tc.tile_pool(name="per", bufs=1) as pp, \
             tc.tile_pool(name="sb", bufs=1) as sbp, \
             tc.tile_pool(name="ps", bufs=1, space="PSUM") as ps:

            # ---- resident weights on the ACT HWDGE queue ----------------
            wq_sb = [wp.tile([128, GD], BF16, tag=f"wq{dc}", name=f"wq{dc}") for dc in range(8)]
            wk_sb = [wp.tile([128, GD], BF16, tag=f"wk{dc}", name=f"wk{dc}") for dc in range(8)]
            wv_sb = [wp.tile([128, GD], BF16, tag=f"wv{dc}", name=f"wv{dc}") for dc in range(8)]
            wo_bf = [wp.tile([128, D], BF16, tag=f"wo{p}", name=f"wo{p}") for p in range(NPAIR)]
            # scalar HWDGE queue: wq then wk (in block-0 use order);
            # sync queue stays free so block-0 x loads go first, then wv/wo
            # are appended behind them (emitted inside the ct==0 iteration).
            for dc in range(8):
                nc.scalar.dma_start(wq_sb[dc][:], wqT[128 * dc:128 * (dc + 1), :])
            for dc in range(8):
                nc.scalar.dma_start(wk_sb[dc][:], wkT[128 * dc:128 * (dc + 1), :])

            # ---- persistent state --------------------------------------
            ident_f = pp.tile([128, 128], F32, tag="ident_f")
            make_identity(nc, ident_f[:])
            ident_bf = pp.tile([128, 128], BF16, tag="ident_bf")
            nc.vector.tensor_copy(ident_bf[:], ident_f[:])
            bv_f32 = pp.tile([66, 64], F32, tag="bv_f32")
            nc.vector.memset(bv_f32[64:65, :], 1.0)
            bvec = pp.tile([66, 64], F32R, tag="bvec")
            nc.vector.tensor_copy(bvec[64:65, :], bv_f32[64:65, :])
            # C/kcum state: f32 master + bf16 matmul operand
            caug_st = [pp.tile([64, 66], F32, tag=f"caug_st{h}", name=f"caug_st{h}")
                       for h in range(HPC)]
            for h in range(HPC):
                nc.gpsimd.memset(caug_st[h][:], 0.0)
                nc.gpsimd.memset(caug_st[h][:, 64:65], 1e-30)
            caug_bf = [pp.tile([128, 66], BF16, tag=f"caug_bf{h}", name=f"caug_bf{h}")
                       for h in range(HPC)]
            for h in range(HPC):
                nc.gpsimd.memset(caug_bf[h][:], 0.0)
                if h % 2 == 0:
                    nc.vector.tensor_copy(caug_bf[h][0:64, :], caug_st[h][:])
            for h in range(1, HPC, 2):
                cbs0 = pp.tile([64, 66], BF16, tag=f"caug_bfs{h}", name=f"cbs0_{h}")
                nc.vector.tensor_copy(cbs0[:], caug_st[h][:])
                nc.gpsimd.dma_start(caug_bf[h][64:128, :], cbs0[:])
            # vaug[s][t4]: [128, 528] bf16, 2 block-parity sets
            vaug = [[pp.tile([128, HPC * 66], BF16, tag=f"vaug{s}_{t}", name=f"vaug{s}_{t}")
                     for t in range(4)] for s in range(2)]
            for s in range(2):
                for t4 in range(4):
                    vv = vaug[s][t4][:].rearrange("p (h c) -> p h c", c=66)
                    nc.gpsimd.memset(vv[:, :, 64:65], 1.0)
                    nc.gpsimd.memset(vv[:, :, 65:66], 0.0)
            # ssb[q][t4]: masked S^T chunks, 4 head-parity sets (h%4), bf16
            ssb = [[pp.tile([128, BT], BF16, tag=f"ssb{q}_{t}", name=f"ssb{q}_{t}")
                    for t in range(4)] for q in range(4)]
            for q in range(4):
                for t4 in range(4):
                    nc.gpsimd.memset(ssb[q][t4][:], 0.0)

            # ---- per-block emission helpers ----------------------------
            HEAD_ORDER = [1, 3, 5, 7, 0, 2, 4, 6]

            def emit_st(h, q4, kt2, qtu2):
                """S^T chunks for head h, restricted query range + masked copies."""
                p, r = h // 2, h % 2
                rb = 64 * r
                for t4 in range(4):
                    c0 = (2 * t4 + 1) * 64
                    c1 = (2 * t4 + 2) * 64
                    pst = ps.tile([128, BT], F32, tag="s", name="pst", bufs=3)
                    nc.tensor.matmul(
                        pst[:, c0:BT],
                        kt2[p][rb:rb + 64, 128 * t4:128 * (t4 + 1)],
                        qtu2[p][rb:rb + 64, c0:BT], start=True, stop=True)
                    flex_copy(ssb[q4][t4][0:64, c0:BT], pst[0:64, c0:BT], BT - c0)
                    if c1 < BT:
                        flex_copy(ssb[q4][t4][64:128, c1:BT], pst[64:128, c1:BT],
                                  BT - c1)

            def emit_out(h, q4, qtu2, par2):
                """natural-orientation context: po_nat [128 tq, 4*66] f32.

                For query chunk j: cols 66j..66j+65 = [out (64) | den | pad].
                inter = qtu2 chunk (K=128, other head's rows killed by the
                zero half of caug_bf) @ caug_bf; intra i<=j = ssb[i] chunk j
                (K=128 keys) @ vaug[i] head cols.  All groups K=128.
                """
                p = h // 2
                po = ps.tile([128, 4 * 66], F32, tag="o", name="po", bufs=2)
                for j in range(4):
                    cj = 66 * j
                    nc.tensor.matmul(po[:, cj:cj + 66],
                                     qtu2[p][:, 128 * j:128 * (j + 1)],
                                     caug_bf[h][:, :], start=True, stop=False)
                    for i in range(j + 1):
                        nc.tensor.matmul(
                            po[:, cj:cj + 66],
                            ssb[q4][i][:, 128 * j:128 * (j + 1)],
                            vaug[par2][i][:, 66 * h:66 * h + 66],
                            start=False, stop=(i == j))
                return po

            def emit_norm(h, po):
                """per-partition normalize: dinv then xo (bf16) per chunk."""
                xo_sb = sbp.tile([128, 256], BF16, tag="xo_sb", name="xo_sb", bufs=4)
                for j in range(4):
                    dvn = sbp.tile([128, 1], F32, tag=f"dvn{j}", name=f"dvn{j}", bufs=4)
                    nc.vector.reciprocal(dvn[:], po[:, 66 * j + 64:66 * j + 65])
                    acc("dve", 130)
                    nc.vector.tensor_scalar_mul(
                        xo_sb[:, 64 * j:64 * (j + 1)],
                        po[:, 66 * j:66 * j + 64], dvn[:])
                    acc("dve", 192)
                return xo_sb

            def emit_xpose(h, xo_sb, xot2):
                """transpose xo chunks into the pair-packed Y operand."""
                p, r = h // 2, h % 2
                if r == 0:
                    dst, db = xot2[p], 0
                else:
                    dst = sbp.tile([64, BT], BF16, tag="xot_o", name="xot_o", bufs=2)
                    db = None
                for j in range(4):
                    ptx = ps.tile([64, 128], BF16, tag="s", name="ptx", bufs=3)
                    nc.tensor.transpose(ptx[:], xo_sb[:, 64 * j:64 * (j + 1)],
                                        ident_bf[:])
                    if r == 0:
                        flex_copy(dst[0:64, 128 * j:128 * (j + 1)], ptx[:], 128)
                    else:
                        flex_copy(dst[:, 128 * j:128 * (j + 1)], ptx[:], 128)
                if r == 1:
                    nc.gpsimd.dma_start(xot2[p][64:128, :], dst[:])

            def emit_cupd(h, ksb, par2, last=False):
                if last:
                    return
                pc = ps.tile([64, 66], F32, tag="c", name="pc", bufs=1)
                for t4 in range(4):
                    nc.tensor.matmul(
                        pc[:], ksb[t4][:, 64 * h:64 * (h + 1)],
                        vaug[par2][t4][:, 66 * h:66 * h + 66],
                        start=(t4 == 0), stop=(t4 == 3))
                nc.vector.tensor_add(caug_st[h][:], caug_st[h][:], pc[:])
                acc("dve", 194)
                if h % 2 == 0:
                    nc.vector.tensor_copy(caug_bf[h][0:64, :], caug_st[h][:])
                    acc("dve", 194)
                else:
                    cbs = pp.tile([64, 66], BF16, tag=f"caug_bfs{h}", name=f"cbs{h}")
                    nc.vector.tensor_copy(cbs[:], caug_st[h][:])
                    acc("dve", 194)
                    nc.gpsimd.dma_start(caug_bf[h][64:128, :], cbs[:])

            # ---- main loop over coarse blocks, software-pipelined ------
            # Iteration ct emits: x loads(ct) + interleaved [attention+Y of
            # block ct-1] and [projections of block ct].  PE then always has
            # independent projection matmuls available while the attention
            # dependency chains (ssb copies, recip/bcast/mul) resolve.
            prev = None
            for ct in range(NBLK + 1):
                proj_units = []
                if ct < NBLK:
                    t0 = ct * BT
                    par2 = ct % 2
                    xsb = [sbp.tile([128, BT], BF16, tag=f"xsb{dc}", name=f"xsb{dc}", bufs=2)
                           for dc in range(8)]
                    for dc in range(8):
                        nc.sync.dma_start(
                            xsb[dc][:], xT[128 * dc:128 * (dc + 1), t0:t0 + BT])
                    if ct == 0:
                        for dc in range(8):
                            nc.sync.dma_start(
                                wv_sb[dc][:], wvT[128 * dc:128 * (dc + 1), :])
                        for p in range(NPAIR):
                            nc.sync.dma_start(
                                wo_bf[p][:], woT[128 * p:128 * (p + 1), :])
                    qtu2 = [sbp.tile([128, BT], BF16, tag=f"qtu{p}", name=f"qtu{p}", bufs=2)
                            for p in range(NPAIR)]
                    ksb = [sbp.tile([128, GD], BF16, tag=f"ksb{t}", name=f"ksb{t}", bufs=2)
                           for t in range(4)]
                    kt2 = [sbp.tile([128, BT], BF16, tag=f"kt{p}", name=f"kt{p}", bufs=2)
                           for p in range(NPAIR)]

                    def mk_q(p, qtu2=qtu2, xsb=xsb):
                        def u():
                            pq = ps.tile([128, BT], F32, tag="proj", name="pq", bufs=2)
                            for dc in range(8):
                                nc.tensor.matmul(
                                    pq[:], wq_sb[dc][:, 128 * p:128 * (p + 1)], xsb[dc][:],
                                    start=(dc == 0), stop=(dc == 7))
                            nc.scalar.activation(qtu2[p][:], pq[:], EXP)
                            acc("act", 612)
                        return u

                    def mk_k(t4, ksb=ksb, xsb=xsb):
                        def u():
                            pk = ps.tile([128, GD], F32, tag="proj", name="pk", bufs=2)
                            for dc in range(8):
                                nc.tensor.matmul(
                                    pk[:], xsb[dc][:, 128 * t4:128 * (t4 + 1)], wk_sb[dc][:],
                                    start=(dc == 0), stop=(dc == 7))
                            nc.scalar.activation(ksb[t4][:], pk[:], EXP)
                            acc("act", 612)
                        return u

                    def mk_tr(t4, ksb=ksb, kt2=kt2):
                        # transpose chunk t4 for ALL pairs (reads only ksb[t4])
                        def u():
                            for p in range(NPAIR):
                                pt = ps.tile([128, 128], BF16, tag="s", name="pt", bufs=3)
                                nc.tensor.transpose(
                                    pt[:], ksb[t4][:, 128 * p:128 * (p + 1)], ident_bf[:])
                                flex_copy(kt2[p][:, 128 * t4:128 * (t4 + 1)], pt[:], 128)
                        return u

                    def mk_v(t4, xsb=xsb, par2=par2):
                        def u():
                            pv = ps.tile([128, GD], F32, tag="proj", name="pv", bufs=2)
                            for dc in range(8):
                                nc.tensor.matmul(
                                    pv[:], xsb[dc][:, 128 * t4:128 * (t4 + 1)], wv_sb[dc][:],
                                    start=(dc == 0), stop=(dc == 7))
                            vv = vaug[par2][t4][:].rearrange("p (h c) -> p h c", c=66)
                            pvv = pv[:].rearrange("p (h c) -> p h c", c=64)
                            flex_copy(vv[:, :, 0:64], pvv[:, :, :], BT)
                        return u

                    proj_units = [mk_q(0), mk_q(1), mk_q(2), mk_q(3),
                                  mk_k(0), mk_tr(0), mk_k(1), mk_tr(1),
                                  mk_k(2), mk_tr(2), mk_k(3), mk_tr(3),
                                  mk_v(0), mk_v(1), mk_v(2), mk_v(3)]
                    cur = dict(t0=t0, par2=par2, qtu2=qtu2, ksb=ksb, kt2=kt2)

                attn_units = []
                if prev is not None:
                    pv_t0, pv_par2 = prev["t0"], prev["par2"]
                    pv_qtu2, pv_ksb, pv_kt2 = prev["qtu2"], prev["ksb"], prev["kt2"]
                    xot2 = [sbp.tile([128, BT], BF16, tag=f"xot{p}", name=f"xot{p}", bufs=2)
                            for p in range(NPAIR)]
                    pend = []

                    def mk_st(i, kt2=pv_kt2, qtu2=pv_qtu2):
                        def u():
                            emit_st(HEAD_ORDER[i], i % 4, kt2, qtu2)
                        return u

                    last_blk = (ct == NBLK)

                    def mk_head(i, qtu2=pv_qtu2, ksb=pv_ksb, kt2=pv_kt2,
                                par2=pv_par2, xot2=xot2, pend=pend, last=last_blk):
                        def u():
                            h = HEAD_ORDER[i]
                            po = emit_out(h, i % 4, qtu2, par2)
                            if i + 4 < 8:
                                emit_st(HEAD_ORDER[i + 4], i % 4, kt2, qtu2)
                            xo_sb = emit_norm(h, po)
                            pend.append((h, xo_sb))
                            if len(pend) > 1:
                                emit_xpose(*pend.pop(0), xot2)
                            emit_cupd(h, ksb, par2, last=last)
                        return u

                    def mk_last_tail(pend=pend, xot2=xot2):
                        def u():
                            emit_xpose(*pend.pop(0), xot2)
                        return u

                    def mk_y(t4, fc, xot2=xot2, t0=pv_t0):
                        def u():
                            py = ps.tile([128, GD], F32, tag="proj", name="py", bufs=2)
                            for p in range(NPAIR):
                                nc.tensor.matmul(
                                    py[:],
                                    xot2[p][:, 128 * t4:128 * (t4 + 1)],
                                    wo_bf[p][:, GD * fc:GD * (fc + 1)],
                                    start=(p == 0), stop=(p == NPAIR - 1))
                            ysb = sbp.tile([128, GD], F32, tag="ysb", name="ysb", bufs=3)
                            flex_copy(ysb[:], py[:], GD)
                            nc.scalar.dma_start(
                                y[t0 + 128 * t4:t0 + 128 * (t4 + 1),
                                  GD * fc:GD * (fc + 1)], ysb[:])
                        return u

                    attn_units = ([mk_st(i) for i in range(4)]
                                  + [mk_head(i) for i in range(8)]
                                  + [mk_last_tail()]
                                  + [mk_y(t4, fc) for t4 in range(4) for fc in range(2)])

                # interleave: attention first (its deps are already met),
                # weaving projection units in proportionally
                na, np_ = len(attn_units), len(proj_units)
                if na == 0:
                    for u in proj_units:
                        u()
                else:
                    pi = 0
                    for k, u in enumerate(attn_units):
                        u()
                        want = (k + 1) * np_ // na
                        while pi < want:
                            proj_units[pi]()
                            pi += 1
                    while pi < np_:
                        proj_units[pi]()
                        pi += 1

                prev = cur if ct < NBLK else None

    nc.compile()
    return nc


def _get_nc():
    if "nc" not in _CACHE:
        _CACHE["nc"] = _build()
    return _CACHE["nc"]


def kernel(x, W_qkv, W_out):
    x = np.asarray(x, dtype=np.float32)
    W_qkv = np.asarray(W_qkv, dtype=np.float32)
    W_out = np.asarray(W_out, dtype=np.float32)
    nc = _get_nc()

    xTs = [np.ascontiguousarray(x[b].T.astype(BF)) for b in range(B)]
    in_maps = []
    for c in range(NC_CORES):
        b, hg = c // 2, c % 2
        s = slice(hg * GD, (hg + 1) * GD)
        in_maps.append({
            "xT": xTs[b],
            "wqT": np.ascontiguousarray(W_qkv[0 * D:1 * D][s].T.astype(BF)),
            "wkT": np.ascontiguousarray(W_qkv[1 * D:2 * D][s].T.astype(BF)),
            "wvT": np.ascontiguousarray(W_qkv[2 * D:3 * D][s].T.astype(BF)),
            "woT": np.ascontiguousarray(W_out[:, s].T.astype(BF)),
        })
    res = run_bass_kernel_spmd(nc, in_maps, core_ids=list(range(NC_CORES)))
    out = np.empty((B, T, D), dtype=np.float32)
    for b in range(B):
        out[b] = res.results[2 * b]["y"] + res.results[2 * b + 1]["y"]
    return out


# revision 16
# speedup vs baseline: 1.0593x; 1.0246x over previous
"""LucidLinearAttention Trainium2 kernel (8-core SPMD), v3.

Sharding: batch b = core//2 (4 batches), head-group hg = core%2 (8 heads each).
Each core: qkv projection for its heads, chunked linear attention over
BT=512 blocks with exact BUCKET=64 causal masking inside the block, partial
output projection. Host sums the two head-group partials per batch.

v3 over v2:
- y stores + weight loads on the Activation HWDGE queue so the SP queue only
  carries x loads: next block's x prefetch is no longer stuck behind the
  current block's y stores (this was a ~4.75us bubble every block).
- Heads processed odds-first so the odd heads' SBUF->SBUF repartition DMAs
  (xot pair packing) complete while the even heads compute.
- S^T emitted 4 heads ahead of the OUT groups; per-head normalize tails
  (bcast/sbb/mul) deferred one head so PE never waits on the recip chain.
- Flexible PSUM->SBUF drains (masked S copies, sbb, ysb) greedily balanced
  across DVE and ACT by estimated cost.

v2 over baseline:
- f32r DRAM tensors, DMA straight into f32r SBUF (no staging copies).
- bf16 attention inner loop (S^T, intra/inter, transposes, C updates).
- Q projection pair-packed (M=128) and Y projection pair-packed (K=128);
  odd heads cross into the packed tiles' rows 64:127 via SBUF->SBUF DMA.
- S^T matmuls restricted to the needed query range per key chunk.
- C/kcum state in f32 (caug_st), re-rounded to bf16 operand each block.
- kcum initialized to 1e-30: den > 0 always, no clamp op needed.
"""
import sys
import ml_dtypes
import numpy as np

BF = ml_dtypes.bfloat16

for p in ("/opt/trn_rl_repo", "/root/.axon_site/_ro/trn_rl_repo"):
    if p not in sys.path:
        sys.path.insert(0, p)

import concourse.mybir as mybir
import concourse.tile as tile
from concourse import bacc
from concourse.bass_utils import run_bass_kernel_spmd
from concourse.masks import make_identity

F32 = mybir.dt.float32
F32R = mybir.dt.float32r
BF16 = mybir.dt.bfloat16
EXP = mybir.ActivationFunctionType.Exp

B, T, D = 4, 4096, 1024
NH, HD, BUCKET = 16, 64, 64
HPC = 8            # heads per core
GD = HPC * HD      # 512 group dim
NBLK = 8           # coarse blocks
BT = T // NBLK     # 512 rows per block
NPAIR = 4
NC_CORES = 8

_CACHE = {}


def _build():
    nc = bacc.Bacc("TRN2", target_bir_lowering=False, debug=False,
                   num_devices=NC_CORES)
    xT = nc.dram_tensor("xT", [D, T], BF16, kind="ExternalInput").ap()
    wqT = nc.dram_tensor("wqT", [D, GD], BF16, kind="ExternalInput").ap()
    wkT = nc.dram_tensor("wkT", [D, GD], BF16, kind="ExternalInput").ap()
    wvT = nc.dram_tensor("wvT", [D, GD], BF16, kind="ExternalInput").ap()
    woT = nc.dram_tensor("woT", [GD, D], BF16, kind="ExternalInput").ap()
    y = nc.dram_tensor("y", [T, D], F32, kind="ExternalOutput").ap()

    # greedy DVE/ACT balance for flexible PSUM->SBUF drains
    eng_acc = {"dve": 0.0, "act": 0.0}

    def flex_copy(dst, src, nfree):
        cd = 125 + 1.042 * nfree
        ca = (143 + 0.833 * nfree) * 1.35
        if eng_acc["dve"] + cd <= eng_acc["act"] + ca:
            eng_acc["dve"] += cd
            nc.vector.tensor_copy(dst, src)
        else:
            eng_acc["act"] += ca
            nc.scalar.copy(dst, src)

    def acc(engine, cost):
        eng_acc[engine] += cost

    with tile.TileContext(nc) as tc:
        with nc.allow_low_precision(reason="f32r/bf16 matmul rounding by design"), \
             tc.tile_pool(name="w", bufs=1) as wp, \
             tc.tile_pool(name="per", bufs=1) as pp, \
             tc.tile_pool(name="sb", bufs=1) as sbp, \
             tc.tile_pool(name="ps", bufs=1, space="PSUM") as ps:

            # ---- resident weights on the ACT HWDGE queue ----------------
            wq_sb = [wp.tile([128, GD], BF16, tag=f"wq{dc}", name=f"wq{dc}") for dc in range(8)]
            wk_sb = [wp.tile([128, GD], BF16, tag=f"wk{dc}", name=f"wk{dc}") for dc in range(8)]
            wv_sb = [wp.tile([128, GD], BF16, tag=f"wv{dc}", name=f"wv{dc}") for dc in range(8)]
            wo_bf = [wp.tile([128, D], BF16, tag=f"wo{p}", name=f"wo{p}") for p in range(NPAIR)]
            # weight queues: wq interleaves with block-0 x on sync (inside
            # the ct==0 iteration, pipelining with Q-proj accumulation order);
            # wk/wv/wo go on the gpsimd SWDGE queue; ACT issues no DMAs at
            # startup so the exps dispatch immediately.
            for dc in range(8):
                nc.gpsimd.dma_start(wk_sb[dc][:], wkT[128 * dc:128 * (dc + 1), :])
            for dc in range(8):
                nc.gpsimd.dma_start(wv_sb[dc][:], wvT[128 * dc:128 * (dc + 1), :])
            for p in range(NPAIR):
                nc.gpsimd.dma_start(wo_bf[p][:], woT[128 * p:128 * (p + 1), :])

            # ---- persistent state --------------------------------------
            ident_f = pp.tile([128, 128], F32, tag="ident_f")
            make_identity(nc, ident_f[:])
            ident_bf = pp.tile([128, 128], BF16, tag="ident_bf")
            nc.vector.tensor_copy(ident_bf[:], ident_f[:])
            bv_f32 = pp.tile([66, 64], F32, tag="bv_f32")
            nc.vector.memset(bv_f32[64:65, :], 1.0)
            bvec = pp.tile([66, 64], F32R, tag="bvec")
            nc.vector.tensor_copy(bvec[64:65, :], bv_f32[64:65, :])
            # C/kcum state: f32 master + bf16 matmul operand
            caug_st = [pp.tile([64, 66], F32, tag=f"caug_st{h}", name=f"caug_st{h}")
                       for h in range(HPC)]
            for h in range(HPC):
                nc.gpsimd.memset(caug_st[h][:], 0.0)
                nc.gpsimd.memset(caug_st[h][:, 64:65], 1e-30)
            caug_bf = [pp.tile([128, 66], BF16, tag=f"caug_bf{h}", name=f"caug_bf{h}")
                       for h in range(HPC)]
            for h in range(HPC):
                nc.gpsimd.memset(caug_bf[h][:], 0.0)
                if h % 2 == 0:
                    nc.vector.tensor_copy(caug_bf[h][0:64, :], caug_st[h][:])
            for h in range(1, HPC, 2):
                cbs0 = pp.tile([64, 66], BF16, tag=f"caug_bfs{h}", name=f"cbs0_{h}")
                nc.vector.tensor_copy(cbs0[:], caug_st[h][:])
                nc.gpsimd.dma_start(caug_bf[h][64:128, :], cbs0[:])
            # vaug[s][t4]: [128, 528] bf16, 2 block-parity sets
            vaug = [[pp.tile([128, HPC * 66], BF16, tag=f"vaug{s}_{t}", name=f"vaug{s}_{t}")
                     for t in range(4)] for s in range(2)]
            for s in range(2):
                for t4 in range(4):
                    vv = vaug[s][t4][:].rearrange("p (h c) -> p h c", c=66)
                    nc.gpsimd.memset(vv[:, :, 64:65], 1.0)
                    nc.gpsimd.memset(vv[:, :, 65:66], 0.0)
            # ssb[q][t4]: masked S^T chunks, 4 head-parity sets (h%4), bf16
            ssb = [[pp.tile([128, BT], BF16, tag=f"ssb{q}_{t}", name=f"ssb{q}_{t}")
                    for t in range(4)] for q in range(4)]
            for q in range(4):
                for t4 in range(4):
                    nc.gpsimd.memset(ssb[q][t4][:], 0.0)

            # ---- per-block emission helpers ----------------------------
            HEAD_ORDER = [1, 3, 5, 7, 0, 2, 4, 6]

            def emit_st(h, q4, kt2, qtu2):
                """S^T chunks for head h, restricted query range + masked copies."""
                p, r = h // 2, h % 2
                rb = 64 * r
                for t4 in range(4):
                    c0 = (2 * t4 + 1) * 64
                    c1 = (2 * t4 + 2) * 64
                    pst = ps.tile([128, BT], F32, tag="s", name="pst", bufs=3)
                    nc.tensor.matmul(
                        pst[:, c0:BT],
                        kt2[p][rb:rb + 64, 128 * t4:128 * (t4 + 1)],
                        qtu2[p][rb:rb + 64, c0:BT], start=True, stop=True)
                    flex_copy(ssb[q4][t4][0:64, c0:BT], pst[0:64, c0:BT], BT - c0)
                    if c1 < BT:
                        flex_copy(ssb[q4][t4][64:128, c1:BT], pst[64:128, c1:BT],
                                  BT - c1)

            def emit_out(h, q4, qtu2, par2):
                """natural-orientation context: po_nat [128 tq, 4*66] f32.

                For query chunk j: cols 66j..66j+65 = [out (64) | den | pad].
                inter = qtu2 chunk (K=128, other head's rows killed by the
                zero half of caug_bf) @ caug_bf; intra i<=j = ssb[i] chunk j
                (K=128 keys) @ vaug[i] head cols.  All groups K=128.
                """
                p = h // 2
                po = ps.tile([128, 4 * 66], F32, tag="o", name="po", bufs=2)
                for j in range(4):
                    cj = 66 * j
                    nc.tensor.matmul(po[:, cj:cj + 66],
                                     qtu2[p][:, 128 * j:128 * (j + 1)],
                                     caug_bf[h][:, :], start=True, stop=False)
                    for i in range(j + 1):
                        nc.tensor.matmul(
                            po[:, cj:cj + 66],
                            ssb[q4][i][:, 128 * j:128 * (j + 1)],
                            vaug[par2][i][:, 66 * h:66 * h + 66],
                            start=False, stop=(i == j))
                return po

            def emit_norm(h, po):
                """per-partition normalize: dinv then xo (bf16) per chunk."""
                xo_sb = sbp.tile([128, 256], BF16, tag="xo_sb", name="xo_sb", bufs=4)
                for j in range(4):
                    dvn = sbp.tile([128, 1], F32, tag=f"dvn{j}", name=f"dvn{j}", bufs=4)
                    nc.vector.reciprocal(dvn[:], po[:, 66 * j + 64:66 * j + 65])
                    acc("dve", 130)
                    nc.vector.tensor_scalar_mul(
                        xo_sb[:, 64 * j:64 * (j + 1)],
                        po[:, 66 * j:66 * j + 64], dvn[:])
                    acc("dve", 192)
                return xo_sb

            def emit_xpose(h, xo_sb, xot2):
                """transpose xo chunks into the pair-packed Y operand."""
                p, r = h // 2, h % 2
                if r == 0:
                    dst, db = xot2[p], 0
                else:
                    dst = sbp.tile([64, BT], BF16, tag="xot_o", name="xot_o", bufs=2)
                    db = None
                for j in range(4):
                    ptx = ps.tile([64, 128], BF16, tag="s", name="ptx", bufs=3)
                    nc.tensor.transpose(ptx[:], xo_sb[:, 64 * j:64 * (j + 1)],
                                        ident_bf[:])
                    if r == 0:
                        flex_copy(dst[0:64, 128 * j:128 * (j + 1)], ptx[:], 128)
                    else:
                        flex_copy(dst[:, 128 * j:128 * (j + 1)], ptx[:], 128)
                if r == 1:
                    nc.gpsimd.dma_start(xot2[p][64:128, :], dst[:])

            def emit_cupd(h, ksb, par2, last=False):
                if last:
                    return
                pc = ps.tile([64, 66], F32, tag="c", name="pc", bufs=1)
                for t4 in range(4):
                    nc.tensor.matmul(
                        pc[:], ksb[t4][:, 64 * h:64 * (h + 1)],
                        vaug[par2][t4][:, 66 * h:66 * h + 66],
                        start=(t4 == 0), stop=(t4 == 3))
                nc.vector.tensor_add(caug_st[h][:], caug_st[h][:], pc[:])
                acc("dve", 194)
                if h % 2 == 0:
                    nc.vector.tensor_copy(caug_bf[h][0:64, :], caug_st[h][:])
                    acc("dve", 194)
                else:
                    cbs = pp.tile([64, 66], BF16, tag=f"caug_bfs{h}", name=f"cbs{h}")
                    nc.vector.tensor_copy(cbs[:], caug_st[h][:])
                    acc("dve", 194)
                    nc.gpsimd.dma_start(caug_bf[h][64:128, :], cbs[:])

            # ---- main loop over coarse blocks, software-pipelined ------
            # Iteration ct emits: x loads(ct) + interleaved [attention+Y of
            # block ct-1] and [projections of block ct].  PE then always has
            # independent projection matmuls available while the attention
            # dependency chains (ssb copies, recip/bcast/mul) resolve.
            prev = None
            for ct in range(NBLK + 1):
                proj_units = []
                if ct < NBLK:
                    t0 = ct * BT
                    par2 = ct % 2
                    xsb = [sbp.tile([128, BT], BF16, tag=f"xsb{dc}", name=f"xsb{dc}", bufs=2)
                           for dc in range(8)]
                    for dc in range(8):
                        nc.sync.dma_start(
                            xsb[dc][:], xT[128 * dc:128 * (dc + 1), t0:t0 + BT])
                        if ct == 0:
                            nc.sync.dma_start(
                                wq_sb[dc][:], wqT[128 * dc:128 * (dc + 1), :])
                    qtu2 = [sbp.tile([128, BT], BF16, tag=f"qtu{p}", name=f"qtu{p}", bufs=2)
                            for p in range(NPAIR)]
                    ksb = [sbp.tile([128, GD], BF16, tag=f"ksb{t}", name=f"ksb{t}", bufs=2)
                           for t in range(4)]
                    kt2 = [sbp.tile([128, BT], BF16, tag=f"kt{p}", name=f"kt{p}", bufs=2)
                           for p in range(NPAIR)]

                    def mk_q(p, qtu2=qtu2, xsb=xsb):
                        def u():
                            pq = ps.tile([128, BT], F32, tag="proj", name="pq", bufs=2)
                            for dc in range(8):
                                nc.tensor.matmul(
                                    pq[:], wq_sb[dc][:, 128 * p:128 * (p + 1)], xsb[dc][:],
                                    start=(dc == 0), stop=(dc == 7))
                            nc.scalar.activation(qtu2[p][:], pq[:], EXP)
                            acc("act", 612)
                        return u

                    def mk_k(t4, ksb=ksb, xsb=xsb):
                        def u():
                            pk = ps.tile([128, GD], F32, tag="proj", name="pk", bufs=2)
                            for dc in range(8):
                                nc.tensor.matmul(
                                    pk[:], xsb[dc][:, 128 * t4:128 * (t4 + 1)], wk_sb[dc][:],
                                    start=(dc == 0), stop=(dc == 7))
                            nc.scalar.activation(ksb[t4][:], pk[:], EXP)
                            acc("act", 612)
                        return u

                    def mk_tr(t4, ksb=ksb, kt2=kt2):
                        # transpose chunk t4 for ALL pairs (reads only ksb[t4])
                        def u():
                            for p in range(NPAIR):
                                pt = ps.tile([128, 128], BF16, tag="s", name="pt", bufs=3)
                                nc.tensor.transpose(
                                    pt[:], ksb[t4][:, 128 * p:128 * (p + 1)], ident_bf[:])
                                flex_copy(kt2[p][:, 128 * t4:128 * (t4 + 1)], pt[:], 128)
                        return u

                    def mk_v(t4, xsb=xsb, par2=par2):
                        def u():
                            pv = ps.tile([128, GD], F32, tag="proj", name="pv", bufs=2)
                            for dc in range(8):
                                nc.tensor.matmul(
                                    pv[:], xsb[dc][:, 128 * t4:128 * (t4 + 1)], wv_sb[dc][:],
                                    start=(dc == 0), stop=(dc == 7))
                            vv = vaug[par2][t4][:].rearrange("p (h c) -> p h c", c=66)
                            pvv = pv[:].rearrange("p (h c) -> p h c", c=64)
                            flex_copy(vv[:, :, 0:64], pvv[:, :, :], BT)
                        return u

                    proj_units = [mk_q(0), mk_q(1), mk_q(2), mk_q(3),
                                  mk_k(0), mk_tr(0), mk_k(1), mk_tr(1),
                                  mk_k(2), mk_tr(2), mk_k(3), mk_tr(3),
                                  mk_v(0), mk_v(1), mk_v(2), mk_v(3)]
                    cur = dict(t0=t0, par2=par2, qtu2=qtu2, ksb=ksb, kt2=kt2)

                attn_units = []
                if prev is not None:
                    pv_t0, pv_par2 = prev["t0"], prev["par2"]
                    pv_qtu2, pv_ksb, pv_kt2 = prev["qtu2"], prev["ksb"], prev["kt2"]
                    xot2 = [sbp.tile([128, BT], BF16, tag=f"xot{p}", name=f"xot{p}", bufs=2)
                            for p in range(NPAIR)]
                    pend = []

                    def mk_st(i, kt2=pv_kt2, qtu2=pv_qtu2):
                        def u():
                            emit_st(HEAD_ORDER[i], i % 4, kt2, qtu2)
                        return u

                    last_blk = (ct == NBLK)

                    def mk_head(i, qtu2=pv_qtu2, ksb=pv_ksb, kt2=pv_kt2,
                                par2=pv_par2, xot2=xot2, pend=pend, last=last_blk):
                        def u():
                            h = HEAD_ORDER[i]
                            po = emit_out(h, i % 4, qtu2, par2)
                            if i + 4 < 8:
                                emit_st(HEAD_ORDER[i + 4], i % 4, kt2, qtu2)
                            xo_sb = emit_norm(h, po)
                            pend.append((h, xo_sb))
                            if len(pend) > 1:
                                emit_xpose(*pend.pop(0), xot2)
                            emit_cupd(h, ksb, par2, last=last)
                        return u

                    def mk_last_tail(pend=pend, xot2=xot2):
                        def u():
                            emit_xpose(*pend.pop(0), xot2)
                        return u

                    def mk_y(t4, fc, xot2=xot2, t0=pv_t0):
                        def u():
                            py = ps.tile([128, GD], F32, tag="proj", name="py", bufs=2)
                            for p in range(NPAIR):
                                nc.tensor.matmul(
                                    py[:],
                                    xot2[p][:, 128 * t4:128 * (t4 + 1)],
                                    wo_bf[p][:, GD * fc:GD * (fc + 1)],
                                    start=(p == 0), stop=(p == NPAIR - 1))
                            ysb = sbp.tile([128, GD], F32, tag="ysb", name="ysb", bufs=3)
                            flex_copy(ysb[:], py[:], GD)
                            nc.scalar.dma_start(
                                y[t0 + 128 * t4:t0 + 128 * (t4 + 1),
                                  GD * fc:GD * (fc + 1)], ysb[:])
                        return u

                    attn_units = ([mk_st(i) for i in range(4)]
                                  + [mk_head(i) for i in range(8)]
                                  + [mk_last_tail()]
                                  + [mk_y(t4, fc) for t4 in range(4) for fc in range(2)])

                # interleave: attention first (its deps are already met),
                # weaving projection units in proportionally
                na, np_ = len(attn_units), len(proj_units)
                if na == 0:
                    for u in proj_units:
                        u()
                else:
                    pi = 0
                    for k, u in enumerate(attn_units):
                        u()
                        want = (k + 1) * np_ // na
                        while pi < want:
                            proj_units[pi]()
                            pi += 1
                    while pi < np_:
                        proj_units[pi]()
                        pi += 1

                prev = cur if ct < NBLK else None

    nc.compile()
    return nc


def _get_nc():
    if "nc" not in _CACHE:
        _CACHE["nc"] = _build()
    return _CACHE["nc"]


def kernel(x, W_qkv, W_out):
    x = np.asarray(x, dtype=np.float32)
    W_qkv = np.asarray(W_qkv, dtype=np.float32)
    W_out = np.asarray(W_out, dtype=np.float32)
    nc = _get_nc()

    xTs = [np.ascontiguousarray(x[b].T.astype(BF)) for b in range(B)]
    in_maps = []
    for c in range(NC_CORES):
        b, hg = c // 2, c % 2
        s = slice(hg * GD, (hg + 1) * GD)
        in_maps.append({
            "xT": xTs[b],
            "wqT": np.ascontiguousarray(W_qkv[0 * D:1 * D][s].T.astype(BF)),
            "wkT": np.ascontiguousarray(W_qkv[1 * D:2 * D][s].T.astype(BF)),
            "wvT": np.ascontiguousarray(W_qkv[2 * D:3 * D][s].T.astype(BF)),
            "woT": np.ascontiguousarray(W_out[:, s].T.astype(BF)),
        })
    res = run_bass_kernel_spmd(nc, in_maps, core_ids=list(range(NC_CORES)))
    out = np.empty((B, T, D), dtype=np.float32)
    for b in range(B):
        out[b] = res.results[2 * b]["y"] + res.results[2 * b + 1]["y"]
    return out
